# revision 1
# baseline (speedup 1.0000x reference)
"""Child-Sum Tree-LSTM (reference.py nn_ChildSumTreeLSTM) on 8 Trainium2
NeuronCores via Bass/Tile, SPMD.

Strategy: everything transposed (features on SBUF partitions, nodes on the
free dimension). Each core owns a contiguous slice of every level (levels
5..8); since children of a node are contiguous, the leaves->level-5
recursion is fully core-local (no collectives). The top levels (4..0,
341 nodes) are finished on the host in numpy during the gather step.

Matmuls run in bf16 (fp32 PSUM accumulation); the forget-gate fx term is
folded into the f-gate matmul via a step-0 broadcast rhs and all biases
ride in the activation instructions. The leaf level is computed in SBUF
groups consumed immediately by level-7 chunks (h/c never touch DRAM).
Emission is software-pipelined for the in-order TensorEngine, and the
child-sum runs incrementally on GpSimd as child chunks complete.
"""
import sys
sys.path.insert(0, '/opt/trn_rl_repo')
import numpy as np
import ml_dtypes
import concourse.bacc as bacc
import concourse.mybir as mybir
from concourse.tile import TileContext
from concourse.alu_op_type import AluOpType

F32 = mybir.dt.float32
BF16 = mybir.dt.bfloat16
AFT = mybir.ActivationFunctionType
P = 128
NCORES = 8
BR = 4


def level_offs(D):
    return [(BR ** l - 1) // (BR - 1) for l in range(D + 1)]


def local_counts(D, cut):
    return {l: BR ** l // NCORES for l in range(cut, D + 1)}


def local_offs(D, cut):
    n = local_counts(D, cut)
    offs = {}
    acc = 0
    for l in range(cut, D + 1):
        offs[l] = acc
        acc += n[l]
    return offs, acc


def build_program(D, cut, chunk=512, c_dtype=BF16, hs_gpsimd=True):
    nloc = local_counts(D, cut)
    loff, total_rows = local_offs(D, cut)
    CDT = c_dtype

    nc = bacc.Bacc("TRN2", target_bir_lowering=False, debug=False,
                   num_devices=NCORES)
    xT = nc.dram_tensor("xT", [2, P, total_rows], BF16, kind="ExternalInput")
    wx = nc.dram_tensor("wx", [2, P, 1024], BF16, kind="ExternalInput")
    wh = nc.dram_tensor("wh", [2, P, 1024], BF16, kind="ExternalInput")
    bias = nc.dram_tensor("bias", [P, 8], F32, kind="ExternalInput")
    ncut = nloc[cut]
    out_h = nc.dram_tensor("out_h", [2, P, ncut], BF16, kind="ExternalOutput")
    out_c = nc.dram_tensor("out_c", [2, P, ncut], CDT, kind="ExternalOutput")

    with TileContext(nc) as tc:
        with tc.tile_pool(name="const", bufs=1) as constp, \
             tc.tile_pool(name="xin", bufs=2) as xin, \
             tc.tile_pool(name="state", bufs=1) as statep, \
             tc.tile_pool(name="leafg", bufs=3) as leafg, \
             tc.tile_pool(name="work", bufs=2) as work, \
             tc.tile_pool(name="psum", bufs=4, space="PSUM") as psum:

            wxt = constp.tile([P, 2, 1024], BF16)
            wht = constp.tile([P, 2, 1024], BF16)
            bt = constp.tile([P, 8], F32)
            nc.sync.dma_start(wxt[:], wx[:].rearrange("a p n -> p a n"))
            nc.sync.dma_start(wht[:], wh[:].rearrange("a p n -> p a n"))
            nc.sync.dma_start(bt[:], bias[:])

            def load_x(l, c0, S, tag="xt", bufs=2):
                t = xin.tile([P, 2, S], BF16, tag=tag, bufs=bufs, name=tag)
                src = xT[:, :, loff[l] + c0: loff[l] + c0 + S]
                nc.sync.dma_start(t[:], src.rearrange("a p n -> p a n"))
                return t

            def gate_tiles(S, pfx=""):
                it = work.tile([P, 2, S], BF16, tag=pfx + "i", name="it")
                ot = work.tile([P, 2, S], BF16, tag=pfx + "o", name="ot")
                ut = work.tile([P, 2, S], BF16, tag=pfx + "u", name="ut")
                return it, ot, ut

            def iou_matmuls(xt, S, hs=None, ptag="ps", pbufs=3):
                """Returns list of 6 psum tiles [P, S] (i0,i1,o0,o1,u0,u1)."""
                out = []
                for mt in range(6):
                    ps = psum.tile([P, S], F32, tag=ptag, bufs=pbufs, name="ps")
                    nc.tensor.matmul(ps[:], wxt[:, 0, mt * P:(mt + 1) * P],
                                     xt[:, 0, :], start=True, stop=False)
                    last = hs is None
                    nc.tensor.matmul(ps[:], wxt[:, 1, mt * P:(mt + 1) * P],
                                     xt[:, 1, :], start=False, stop=last)
                    if hs is not None:
                        nc.tensor.matmul(ps[:], wht[:, 0, mt * P:(mt + 1) * P],
                                         hs[:, 0, :], start=False, stop=False)
                        nc.tensor.matmul(ps[:], wht[:, 1, mt * P:(mt + 1) * P],
                                         hs[:, 1, :], start=False, stop=True)
                    out.append(ps)
                return out

            def gates_from_psums(iou, it, ot, ut, S):
                for ft in range(2):
                    nc.scalar.activation(it[:, ft, :], iou[ft][:], AFT.Sigmoid,
                                         bias=bt[:, ft:ft + 1])
                    nc.scalar.activation(ot[:, ft, :], iou[2 + ft][:], AFT.Sigmoid,
                                         bias=bt[:, 2 + ft:3 + ft])
                    nc.scalar.activation(ut[:, ft, :], iou[4 + ft][:], AFT.Tanh,
                                         bias=bt[:, 4 + ft:5 + ft])

            def leaf_chunk(xt, S, h_dst, c_dst):
                iou = iou_matmuls(xt, S, ptag="psl")
                it, ot, ut = gate_tiles(S, pfx="l")
                gates_from_psums(iou, it, ot, ut, S)
                # fused over both ftiles
                with nc.allow_low_precision(reason="bf16 by design"):
                    nc.vector.tensor_tensor(c_dst, it[:], ut[:], AluOpType.mult)
                    nc.scalar.activation(ut[:], c_dst, AFT.Tanh)
                    nc.vector.tensor_tensor(h_dst, ot[:], ut[:], AluOpType.mult)

            def internal_chunk(l, c0, S, ch_h, ch_c, h_dst, c_dst, hs):
                xt = load_x(l, c0, S)
                # forget gates first: fh@child_h + fx@x_parent (broadcast rhs)
                nch = BR * S
                fw = min(1024, nch)          # f-psum width (<=2 banks)
                ft_tile = work.tile([P, 2, nch], BF16, tag="f", name="ft_tile")
                for ftt in range(2):
                    for q in range(nch // fw):
                        psf = psum.tile([P, fw], F32, tag="psf", bufs=1,
                                        name="psf")
                        for half in range(fw // 512) or [0]:
                            lo = q * fw + half * 512
                            w_ = min(512, nch - lo)
                            dst = psf[:, half * 512: half * 512 + w_]
                            nc.tensor.matmul(
                                dst, wht[:, 0, (768 + ftt * P):(768 + (ftt + 1) * P)],
                                ch_h[:, 0, lo:lo + w_], start=True, stop=False)
                            nc.tensor.matmul(
                                dst, wht[:, 1, (768 + ftt * P):(768 + (ftt + 1) * P)],
                                ch_h[:, 1, lo:lo + w_], start=False, stop=False)
                            plo, pw = lo // BR, w_ // BR
                            for kt in range(2):
                                rhs = xt[:, kt, plo:plo + pw] \
                                    .rearrange("p (n b) -> p n b", b=1) \
                                    .broadcast_to([P, pw, BR])
                                nc.tensor.matmul(
                                    dst.rearrange("p (n b) -> p n b", b=BR),
                                    wxt[:, kt, (768 + ftt * P):(768 + (ftt + 1) * P)],
                                    rhs, start=False, stop=(kt == 1))
                        nc.scalar.activation(ft_tile[:, ftt, q * fw:(q + 1) * fw],
                                             psf[:], AFT.Sigmoid,
                                             bias=bt[:, 6 + ftt:7 + ftt])
                # f * c_child (in place), group-sum into fcs
                fcs = work.tile([P, 2, S], CDT, tag="fcs", name="fcs")
                with nc.allow_low_precision(reason="bf16 by design"):
                    nc.vector.tensor_tensor(ft_tile[:], ft_tile[:], ch_c,
                                            AluOpType.mult)
                    for ft in range(2):
                        nc.vector.tensor_reduce(
                            fcs[:, ft, :],
                            ft_tile[:, ft, :].rearrange("p (n b) -> p n b", b=BR),
                            mybir.AxisListType.X, AluOpType.add)
                iou = iou_matmuls(xt, S, hs)
                it, ot, ut = gate_tiles(S)
                gates_from_psums(iou, it, ot, ut, S)
                with nc.allow_low_precision(reason="bf16 by design"):
                    # c = i*u + fcs ; h = o * tanh(c)   (ftile-fused)
                    nc.vector.tensor_tensor(it[:], it[:], ut[:], AluOpType.mult)
                    nc.vector.tensor_tensor(c_dst, it[:], fcs[:], AluOpType.add)
                    nc.scalar.activation(ut[:], c_dst, AFT.Tanh)
                    nc.vector.tensor_tensor(h_dst, ot[:], ut[:], AluOpType.mult)

            # ---- persistent level tiles ----
            lt_h = {}
            lt_c = {}
            for l in range(cut, D):
                lt_h[l] = statep.tile([P, 2, nloc[l]], BF16, tag=f"h{l}",
                                      name=f"h{l}")
                lt_c[l] = statep.tile([P, 2, nloc[l]], CDT, tag=f"c{l}",
                                      name=f"c{l}")
            # child-sum accumulators, filled incrementally as child h completes
            hs_t = {}
            for l in range(cut, D):
                hs_t[l] = statep.tile([P, 2, nloc[l]], BF16, tag=f"hs{l}",
                                      name=f"hs{l}")

            def emit_hsum(lpar, ch_ap, c0p, Sp):
                """Sum 4-child groups of ch_ap ([P,2,4*Sp]) into
                hs_t[lpar][:, :, c0p:c0p+Sp]."""
                with nc.allow_low_precision(reason="bf16 by design"):
                    htmp = work.tile([P, 2, Sp, 2], BF16, tag="htmp", name="htmp")
                    for ft in range(2):
                        v = ch_ap[:, ft, :].rearrange("p (n b) -> p n b", b=BR)
                        nc.gpsimd.tensor_add(htmp[:, ft, :, :],
                                             v[:, :, 0:2], v[:, :, 2:4])
                        nc.gpsimd.tensor_add(hs_t[lpar][:, ft, c0p:c0p + Sp],
                                             htmp[:, ft, :, 0],
                                             htmp[:, ft, :, 1])

            # ---- leaf level fused with level D-1 ----
            lp = D - 1
            pc = min(chunk, nloc[lp])
            n_groups = nloc[lp] // pc
            leafc = pc * BR
            pending = None
            for g in range(n_groups):
                h8g = leafg.tile([P, 2, leafc], BF16, tag="h8g", name="h8g")
                c8g = leafg.tile([P, 2, leafc], CDT, tag="c8g", name="c8g")
                lsub = min(chunk, leafc)
                for s in range(leafc // lsub):
                    xt = load_x(D, g * leafc + s * lsub, lsub, tag="xleaf",
                                bufs=4)
                    hsl = h8g[:, :, s * lsub:(s + 1) * lsub]
                    leaf_chunk(xt, lsub, hsl,
                               c8g[:, :, s * lsub:(s + 1) * lsub])
                    emit_hsum(lp, hsl, g * pc + s * lsub // BR, lsub // BR)
                if pending is not None:
                    internal_chunk(*pending)
                pending = (lp, g * pc, pc, h8g[:], c8g[:],
                           lt_h[lp][:, :, g * pc:(g + 1) * pc],
                           lt_c[lp][:, :, g * pc:(g + 1) * pc],
                           hs_t[lp][:, :, g * pc:(g + 1) * pc])
            internal_chunk(*pending)
            # ---- levels D-2 .. cut ----
            for l in range(D - 2, cut - 1, -1):
                # child-sum for this level's parents from level l+1 h
                emit_hsum(l, lt_h[l + 1][:], 0, nloc[l])
                S = nloc[l]
                pcS = min(chunk, S)
                for c0 in range(0, S, pcS):
                    internal_chunk(l, c0, pcS,
                                   lt_h[l + 1][:, :, c0 * BR:(c0 + pcS) * BR],
                                   lt_c[l + 1][:, :, c0 * BR:(c0 + pcS) * BR],
                                   lt_h[l][:, :, c0:c0 + pcS],
                                   lt_c[l][:, :, c0:c0 + pcS],
                                   hs_t[l][:, :, c0:c0 + pcS])

            nc.sync.dma_start(out_h[:].rearrange("a p n -> p a n"), lt_h[cut][:])
            nc.sync.dma_start(out_c[:].rearrange("a p n -> p a n"), lt_c[cut][:])

    nc.compile()
    return nc


def shard_inputs(x, W_iou_x, b_iou_x, W_iou_h, b_iou_h, W_fx, b_fx, W_fh, b_fh,
                 D, cut):
    offs = level_offs(D)
    nloc = local_counts(D, cut)
    wx_cat = np.concatenate([W_iou_x, W_fx], axis=0)
    wh_cat = np.concatenate([W_iou_h, W_fh], axis=0)
    wx_d = np.ascontiguousarray(wx_cat.T).reshape(2, P, 1024).astype(ml_dtypes.bfloat16)
    wh_d = np.ascontiguousarray(wh_cat.T).reshape(2, P, 1024).astype(ml_dtypes.bfloat16)
    b_iou = (b_iou_x + b_iou_h).reshape(6, P).T
    b_f = (b_fx + b_fh).reshape(2, P).T
    bias = np.ascontiguousarray(
        np.concatenate([b_iou, b_f], axis=1)).astype(np.float32)
    in_maps = []
    for k in range(NCORES):
        rows = []
        for l in range(cut, D + 1):
            n = nloc[l]
            rows.append(x[offs[l] + k * n: offs[l] + (k + 1) * n])
        xl = np.concatenate(rows, axis=0)
        xTk = np.ascontiguousarray(xl.T).reshape(2, P, -1).astype(ml_dtypes.bfloat16)
        in_maps.append({"xT": xTk, "wx": wx_d, "wh": wh_d, "bias": bias})
    return in_maps


def finish_host(results, x, W_iou_x, b_iou_x, W_iou_h, b_iou_h,
                W_fx, b_fx, W_fh, b_fh, D, cut):
    ncut = BR ** cut
    npc = ncut // NCORES
    Hc = np.empty((ncut, 256), np.float32)
    Cc = np.empty((ncut, 256), np.float32)
    for k in range(NCORES):
        oh = results[k]["out_h"].astype(np.float32).reshape(256, npc)
        oc = results[k]["out_c"].astype(np.float32).reshape(256, npc)
        Hc[k * npc:(k + 1) * npc] = oh.T
        Cc[k * npc:(k + 1) * npc] = oc.T
    sig = lambda v: 1.0 / (1.0 + np.exp(-v))
    h_next, c_next = Hc, Cc
    for l in range(cut - 1, -1, -1):
        n, off = BR ** l, (BR ** l - 1) // 3
        xl = x[off:off + n]
        child_h = h_next.reshape(n, BR, 256)
        child_c = c_next.reshape(n, BR, 256)
        chs = child_h.sum(axis=1)
        iou = xl @ W_iou_x.T + b_iou_x + chs @ W_iou_h.T + b_iou_h
        i, o, u = np.split(iou, 3, axis=1)
        i, o, u = sig(i), sig(o), np.tanh(u)
        f = sig(child_h @ W_fh.T + b_fh + (xl @ W_fx.T + b_fx)[:, None, :])
        c = i * u + (f * child_c).sum(axis=1)
        h = o * np.tanh(c)
        h_next, c_next = h, c
    return c_next.astype(np.float32), h_next.astype(np.float32)


# ---------------- public API ----------------

_D = 8
_CUT = 5
_CACHE = {}


def _get_program():
    if "nc" not in _CACHE:
        _CACHE["nc"] = build_program(_D, _CUT)
    return _CACHE["nc"]


def kernel(x, W_iou_x, b_iou_x, W_iou_h, b_iou_h, W_fx, b_fx, W_fh, b_fh):
    from concourse import bass_utils
    x = np.asarray(x, dtype=np.float32)
    args = [np.asarray(a, dtype=np.float32) for a in
            (W_iou_x, b_iou_x, W_iou_h, b_iou_h, W_fx, b_fx, W_fh, b_fh)]
    nc = _get_program()
    in_maps = shard_inputs(x, *args, _D, _CUT)
    res = bass_utils.run_bass_kernel_spmd(nc, in_maps,
                                          core_ids=list(range(NCORES)))
    c, h = finish_host(res.results, x, *args, _D, _CUT)
    return c, h



# revision 3
# speedup vs baseline: 1.1426x; 1.1426x over previous
"""Child-Sum Tree-LSTM (reference.py nn_ChildSumTreeLSTM) on 8 Trainium2
NeuronCores via Bass/Tile, SPMD.

Strategy: everything transposed (features on SBUF partitions, nodes on the
free dimension). Each core owns a contiguous slice of every level (levels
cut..8); since children of a node are contiguous, the leaves->level-cut
recursion is fully core-local (no collectives). The top levels (cut-1..0,
few nodes) are finished on the host in numpy during the gather step.

Key scheduling ideas vs the naive version:
- PSUM budget: per-gate [P,2,S] f32 psum tiles (i/o/u = 3 tags x 2 banks)
  plus a [P,1024] f-gate tag (2 banks) = exactly 8 banks, so TensorE can
  fill one gate's psum while ScalarE drains another.
- Emission interleaves ACT-heavy leaf chunks with tensor-heavy internal
  (level D-1) chunk slices, so the in-order per-engine queues always have
  runnable work for both TensorE and ScalarE.
- Child sums and the f*c group reduction run as 2x-mode tensor_tensor add
  trees on DVE (tensor_reduce only has a 1x uop); leaf child-sums alternate
  DVE/GpSimd to keep both off the critical path.
- Matmuls in bf16 (fp32 PSUM), biases ride the activation instructions.
"""
import sys
sys.path.insert(0, '/opt/trn_rl_repo')
import numpy as np
import ml_dtypes
import concourse.bacc as bacc
import concourse.mybir as mybir
from concourse.tile import TileContext
from concourse.alu_op_type import AluOpType

F32 = mybir.dt.float32
BF16 = mybir.dt.bfloat16
AFT = mybir.ActivationFunctionType
P = 128
NCORES = 8
BR = 4


def level_offs(D):
    return [(BR ** l - 1) // (BR - 1) for l in range(D + 1)]


def local_counts(D, cut):
    return {l: BR ** l // NCORES for l in range(cut, D + 1)}


def local_offs(D, cut):
    n = local_counts(D, cut)
    offs = {}
    acc = 0
    for l in range(cut, D + 1):
        offs[l] = acc
        acc += n[l]
    return offs, acc


def build_program(D, cut, chunk=512, c_dtype=BF16):
    nloc = local_counts(D, cut)
    loff, total_rows = local_offs(D, cut)
    CDT = c_dtype
    S = chunk

    nc = bacc.Bacc("TRN2", target_bir_lowering=False, debug=False,
                   num_devices=NCORES)
    xT = nc.dram_tensor("xT", [2, P, total_rows], BF16, kind="ExternalInput")
    wx = nc.dram_tensor("wx", [2, P, 1024], BF16, kind="ExternalInput")
    wh = nc.dram_tensor("wh", [2, P, 1024], BF16, kind="ExternalInput")
    bias = nc.dram_tensor("bias", [P, 8], F32, kind="ExternalInput")
    ncut = nloc[cut]
    out_h = nc.dram_tensor("out_h", [2, P, ncut], BF16, kind="ExternalOutput")
    out_c = nc.dram_tensor("out_c", [2, P, ncut], CDT, kind="ExternalOutput")

    with TileContext(nc) as tc:
        with tc.tile_pool(name="const", bufs=1) as constp, \
             tc.tile_pool(name="xin", bufs=2) as xin, \
             tc.tile_pool(name="state", bufs=1) as statep, \
             tc.tile_pool(name="leafg", bufs=3) as leafg, \
             tc.tile_pool(name="work", bufs=3) as work, \
             tc.tile_pool(name="psum", bufs=1, space="PSUM") as psum:

            wxt = constp.tile([P, 2, 1024], BF16)
            wht = constp.tile([P, 2, 1024], BF16)
            bt = constp.tile([P, 8], F32)
            nc.sync.dma_start(wxt[:], wx[:].rearrange("a p n -> p a n"))
            nc.sync.dma_start(wht[:], wh[:].rearrange("a p n -> p a n"))
            nc.sync.dma_start(bt[:], bias[:])

            def load_x(l, c0, Sx, tag="xt", bufs=2):
                t = xin.tile([P, 2, Sx], BF16, tag=tag, bufs=bufs, name=tag)
                src = xT[:, :, loff[l] + c0: loff[l] + c0 + Sx]
                nc.sync.dma_start(t[:], src.rearrange("a p n -> p a n"))
                return t

            # ---- persistent level tiles ----
            lt_h = {}
            lt_c = {}
            for l in range(cut, D):
                lt_h[l] = statep.tile([P, 2, nloc[l]], BF16, tag=f"h{l}",
                                      name=f"h{l}")
                lt_c[l] = statep.tile([P, 2, nloc[l]], CDT, tag=f"c{l}",
                                      name=f"c{l}")
            # child-sum accumulators, filled as child h chunks complete
            hs_t = {}
            for l in range(cut, D):
                hs_t[l] = statep.tile([P, 2, nloc[l]], BF16, tag=f"hs{l}",
                                      name=f"hs{l}")

            def emit_hsum(lpar, ch_ap, c0p, Sp, eng):
                """Sum 4-child groups of ch_ap ([P,2,4*Sp]) into
                hs_t[lpar][:, :, c0p:c0p+Sp] with a 2-level add tree."""
                with nc.allow_low_precision(reason="bf16 by design"):
                    htmp = work.tile([P, 2, Sp, 2], BF16, tag="htmp",
                                     bufs=2, name="htmp")
                    for ft in range(2):
                        v = ch_ap[:, ft, :].rearrange("p (n b) -> p n b", b=BR)
                        eng.tensor_tensor(htmp[:, ft, :, :],
                                          v[:, :, 0:2], v[:, :, 2:4],
                                          AluOpType.add)
                        eng.tensor_tensor(hs_t[lpar][:, ft, c0p:c0p + Sp],
                                          htmp[:, ft, :, 0],
                                          htmp[:, ft, :, 1],
                                          AluOpType.add)

            def iou_mms(xt, Sx, hs=None):
                """Per-gate psum tiles [P, 2, Sx] (tags gi/go/gu, 2 banks
                each).  Returns [ps_i, ps_o, ps_u]."""
                out = []
                for gname, gidx in (("gi", 0), ("go", 1), ("gu", 2)):
                    ps = psum.tile([P, 2, Sx], F32, tag=gname, bufs=1,
                                   name=gname)
                    for ft in range(2):
                        mt = gidx * 2 + ft
                        dst = ps[:, ft, :]
                        nc.tensor.matmul(dst, wxt[:, 0, mt * P:(mt + 1) * P],
                                         xt[:, 0, :], start=True, stop=False)
                        last = hs is None
                        nc.tensor.matmul(dst, wxt[:, 1, mt * P:(mt + 1) * P],
                                         xt[:, 1, :], start=False, stop=last)
                        if hs is not None:
                            nc.tensor.matmul(dst,
                                             wht[:, 0, mt * P:(mt + 1) * P],
                                             hs[:, 0, :], start=False,
                                             stop=False)
                            nc.tensor.matmul(dst,
                                             wht[:, 1, mt * P:(mt + 1) * P],
                                             hs[:, 1, :], start=False,
                                             stop=True)
                    out.append(ps)
                return out

            def gate_acts(iou, Sx):
                it = work.tile([P, 2, Sx], BF16, tag="it", name="it")
                ot = work.tile([P, 2, Sx], BF16, tag="ot", name="ot")
                ut = work.tile([P, 2, Sx], BF16, tag="ut", name="ut")
                for gidx, (dst, fn) in enumerate(
                        ((it, AFT.Sigmoid), (ot, AFT.Sigmoid),
                         (ut, AFT.Tanh))):
                    for ft in range(2):
                        mt = gidx * 2 + ft
                        nc.scalar.activation(dst[:, ft, :],
                                             iou[gidx][:, ft, :], fn,
                                             bias=bt[:, mt:mt + 1])
                return it, ot, ut

            def leaf_chunk(g, k, h8g, c8g, hsum_eng):
                c0 = (g * BR + k) * S
                xt = load_x(D, c0, S, tag="xleaf", bufs=4)
                iou = iou_mms(xt, S)
                it, ot, ut = gate_acts(iou, S)
                h_dst = h8g[:, :, k * S:(k + 1) * S]
                c_dst = c8g[:, :, k * S:(k + 1) * S]
                with nc.allow_low_precision(reason="bf16 by design"):
                    nc.vector.tensor_tensor(c_dst, it[:], ut[:],
                                            AluOpType.mult)
                    nc.scalar.activation(ut[:], c_dst, AFT.Tanh)
                    nc.vector.tensor_tensor(h_dst, ot[:], ut[:],
                                            AluOpType.mult)
                emit_hsum(D - 1, h_dst, g * S + k * (S // BR), S // BR,
                          hsum_eng)

            # ---- internal chunk, split into 4 emission slices ----
            def int_f_half(st, ftt):
                """f-gate pre-acts + sigmoid + f*c + group-sum for one
                feature tile, in psum rounds of <=1024 columns."""
                Sp = st["Sp"]
                nch = BR * Sp
                ch_h, ch_c, xt = st["ch_h"], st["ch_c"], st["xt"]
                ft_tile, fcs = st["ft_tile"], st["fcs"]
                woff = 768 + ftt * P
                rw = min(1024, nch)
                for r in range(nch // rw):
                    pf = psum.tile([P, rw], F32, tag="pf", bufs=1, name="pf")
                    for q in range(rw // 512) or [0]:
                        lo = r * rw + q * 512
                        w_ = min(512, nch - lo)
                        dst = pf[:, q * 512:q * 512 + w_]
                        nc.tensor.matmul(dst, wht[:, 0, woff:woff + P],
                                         ch_h[:, 0, lo:lo + w_],
                                         start=True, stop=False)
                        nc.tensor.matmul(dst, wht[:, 1, woff:woff + P],
                                         ch_h[:, 1, lo:lo + w_],
                                         start=False, stop=False)
                        plo, pw = lo // BR, w_ // BR
                        for kt in range(2):
                            rhs = xt[:, kt, plo:plo + pw] \
                                .rearrange("p (n b) -> p n b", b=1) \
                                .broadcast_to([P, pw, BR])
                            nc.tensor.matmul(
                                dst.rearrange("p (n b) -> p n b", b=BR),
                                wxt[:, kt, woff:woff + P],
                                rhs, start=False, stop=(kt == 1))
                    nc.scalar.activation(
                        ft_tile[:, ftt, r * rw:(r + 1) * rw], pf[:],
                        AFT.Sigmoid, bias=bt[:, 6 + ftt:7 + ftt])
                with nc.allow_low_precision(reason="bf16 by design"):
                    # f * c (in place), then 4-child group sum into fcs
                    nc.vector.tensor_tensor(ft_tile[:, ftt, :],
                                            ft_tile[:, ftt, :],
                                            ch_c[:, ftt, :], AluOpType.mult)
                    v = ft_tile[:, ftt, :].rearrange("p (n b) -> p n b", b=BR)
                    ftmp = work.tile([P, Sp, 2], BF16, tag="ftmp", bufs=2,
                                     name="ftmp")
                    nc.vector.tensor_tensor(ftmp[:], v[:, :, 0:2],
                                            v[:, :, 2:4], AluOpType.add)
                    nc.vector.tensor_tensor(fcs[:, ftt, :], ftmp[:, :, 0],
                                            ftmp[:, :, 1], AluOpType.add)

            def int_iou(st):
                st["iou"] = iou_mms(st["xt"], st["Sp"], st["hs"])
                st["gates"] = gate_acts(st["iou"], st["Sp"])

            def int_fin(st):
                it, ot, ut = st["gates"]
                fcs = st["fcs"]
                lv = st["lv"]
                with nc.allow_low_precision(reason="bf16 by design"):
                    nc.vector.tensor_tensor(it[:], it[:], ut[:],
                                            AluOpType.mult)
                    nc.vector.tensor_tensor(st["c_dst"], it[:], fcs[:],
                                            AluOpType.add)
                    nc.scalar.activation(ut[:], st["c_dst"], AFT.Tanh)
                    nc.vector.tensor_tensor(st["h_dst"], ot[:], ut[:],
                                            AluOpType.mult)
                if lv - 1 >= cut:
                    emit_hsum(lv - 1, st["h_dst"], st["c0"] // BR,
                              st["Sp"] // BR, nc.vector)

            def int_slice(st, k):
                if k == 0:
                    int_f_half(st, 0)
                elif k == 1:
                    int_f_half(st, 1)
                elif k == 2:
                    int_iou(st)
                else:
                    int_fin(st)

            def make_state(lv, c0, Sp, ch_h, ch_c):
                xt = load_x(lv, c0, Sp, tag="xi", bufs=2)
                ft_tile = work.tile([P, 2, BR * Sp], BF16, tag="ftile",
                                    bufs=2, name="ft_tile")
                fcs = work.tile([P, 2, Sp], BF16, tag="fcs", bufs=2,
                                name="fcs")
                return {"lv": lv, "c0": c0, "Sp": Sp, "xt": xt,
                        "ch_h": ch_h, "ch_c": ch_c, "ft_tile": ft_tile,
                        "fcs": fcs, "hs": hs_t[lv][:, :, c0:c0 + Sp],
                        "h_dst": lt_h[lv][:, :, c0:c0 + Sp],
                        "c_dst": lt_c[lv][:, :, c0:c0 + Sp]}

            # ---- leaf groups fused+interleaved with level D-1 ----
            lp = D - 1
            n_lg = nloc[D] // (BR * S)     # leaf groups of BR*S leaves
            pend = None
            for g in range(n_lg):
                h8g = leafg.tile([P, 2, BR * S], BF16, tag="h8g", name="h8g")
                c8g = leafg.tile([P, 2, BR * S], CDT, tag="c8g", name="c8g")
                for k in range(BR):
                    leaf_chunk(g, k, h8g, c8g,
                               nc.gpsimd if k % 2 else nc.vector)
                    if pend is not None:
                        int_slice(pend, k)
                pend = make_state(lp, g * S, S, h8g[:], c8g[:])
            for k in range(BR):
                int_slice(pend, k)

            # ---- levels D-2 .. cut ----
            for lv in range(D - 2, cut - 1, -1):
                Sp = min(S, nloc[lv])
                for c0 in range(0, nloc[lv], Sp):
                    st = make_state(lv, c0, Sp,
                                    lt_h[lv + 1][:, :, c0 * BR:(c0 + Sp) * BR],
                                    lt_c[lv + 1][:, :, c0 * BR:(c0 + Sp) * BR])
                    for k in range(BR):
                        int_slice(st, k)

            nc.sync.dma_start(out_h[:].rearrange("a p n -> p a n"),
                              lt_h[cut][:])
            nc.sync.dma_start(out_c[:].rearrange("a p n -> p a n"),
                              lt_c[cut][:])

    nc.compile()
    return nc


def shard_inputs(x, W_iou_x, b_iou_x, W_iou_h, b_iou_h, W_fx, b_fx, W_fh, b_fh,
                 D, cut):
    offs = level_offs(D)
    nloc = local_counts(D, cut)
    wx_cat = np.concatenate([W_iou_x, W_fx], axis=0)
    wh_cat = np.concatenate([W_iou_h, W_fh], axis=0)
    wx_d = np.ascontiguousarray(wx_cat.T).reshape(2, P, 1024).astype(ml_dtypes.bfloat16)
    wh_d = np.ascontiguousarray(wh_cat.T).reshape(2, P, 1024).astype(ml_dtypes.bfloat16)
    b_iou = (b_iou_x + b_iou_h).reshape(6, P).T
    b_f = (b_fx + b_fh).reshape(2, P).T
    bias = np.ascontiguousarray(
        np.concatenate([b_iou, b_f], axis=1)).astype(np.float32)
    in_maps = []
    for k in range(NCORES):
        rows = []
        for l in range(cut, D + 1):
            n = nloc[l]
            rows.append(x[offs[l] + k * n: offs[l] + (k + 1) * n])
        xl = np.concatenate(rows, axis=0)
        xTk = np.ascontiguousarray(xl.T).reshape(2, P, -1).astype(ml_dtypes.bfloat16)
        in_maps.append({"xT": xTk, "wx": wx_d, "wh": wh_d, "bias": bias})
    return in_maps


def finish_host(results, x, W_iou_x, b_iou_x, W_iou_h, b_iou_h,
                W_fx, b_fx, W_fh, b_fh, D, cut):
    ncut = BR ** cut
    npc = ncut // NCORES
    Hc = np.empty((ncut, 256), np.float32)
    Cc = np.empty((ncut, 256), np.float32)
    for k in range(NCORES):
        oh = results[k]["out_h"].astype(np.float32).reshape(256, npc)
        oc = results[k]["out_c"].astype(np.float32).reshape(256, npc)
        Hc[k * npc:(k + 1) * npc] = oh.T
        Cc[k * npc:(k + 1) * npc] = oc.T
    sig = lambda v: 1.0 / (1.0 + np.exp(-v))
    h_next, c_next = Hc, Cc
    for l in range(cut - 1, -1, -1):
        n, off = BR ** l, (BR ** l - 1) // 3
        xl = x[off:off + n]
        child_h = h_next.reshape(n, BR, 256)
        child_c = c_next.reshape(n, BR, 256)
        chs = child_h.sum(axis=1)
        iou = xl @ W_iou_x.T + b_iou_x + chs @ W_iou_h.T + b_iou_h
        i, o, u = np.split(iou, 3, axis=1)
        i, o, u = sig(i), sig(o), np.tanh(u)
        f = sig(child_h @ W_fh.T + b_fh + (xl @ W_fx.T + b_fx)[:, None, :])
        c = i * u + (f * child_c).sum(axis=1)
        h = o * np.tanh(c)
        h_next, c_next = h, c
    return c_next.astype(np.float32), h_next.astype(np.float32)


# ---------------- public API ----------------

_D = 8
_CUT = 6
_CACHE = {}


def _get_program():
    if "nc" not in _CACHE:
        _CACHE["nc"] = build_program(_D, _CUT)
    return _CACHE["nc"]


def kernel(x, W_iou_x, b_iou_x, W_iou_h, b_iou_h, W_fx, b_fx, W_fh, b_fh):
    from concourse import bass_utils
    x = np.asarray(x, dtype=np.float32)
    args = [np.asarray(a, dtype=np.float32) for a in
            (W_iou_x, b_iou_x, W_iou_h, b_iou_h, W_fx, b_fx, W_fh, b_fh)]
    nc = _get_program()
    in_maps = shard_inputs(x, *args, _D, _CUT)
    res = bass_utils.run_bass_kernel_spmd(nc, in_maps,
                                          core_ids=list(range(NCORES)))
    c, h = finish_host(res.results, x, *args, _D, _CUT)
    return c, h
